# revision 5
# baseline (speedup 1.0000x reference)
"""Causal depthwise conv1d (B=8, L=4096, C=1024, K=7) on 8 Trainium2 cores.

Strategy:
  - Pure data parallel: one batch element per NeuronCore.
  - Host casts x to fp16 (11-bit mantissa; rel err ~1e-4 level) and builds
    per-channel-group diagonal weight matrices so the depthwise conv becomes
    7 accumulating diagonal matmuls on the TensorEngine (PSUM fp32 accum).
  - Device layout: channels-on-partitions via DMA xbar transpose (fp16),
    conv along the free (L) dim with causal zero padding, ScalarE drains
    PSUM with per-partition bias, xbar transpose back, fp16 store.
  - Host casts fp16 result back to fp32.
"""

import os
import sys

import numpy as np

sys.path.insert(0, "/opt/trn_rl_repo")

B, L, C, K = 8, 4096, 1024, 7
G = C // 128            # channel groups of 128 partitions
PAD = 16                # left zero pad (>= K-1, 32B aligned for xbar dest)
NCHUNK = 512            # matmul free-dim chunk (one PSUM fp32 bank)
HALF = 2048             # L is processed in halves per group (PSUM capacity)

_CACHE: dict = {}
LAST_RESULTS = None     # BassKernelResults of the most recent run (for test.py)


def _build_device_program():
    import concourse.bacc as bacc
    import concourse.mybir as mybir
    from concourse.tile import TileContext

    fp16 = mybir.dt.float16
    fp32 = mybir.dt.float32

    nc = bacc.Bacc(
        "TRN2",
        target_bir_lowering=False,
        debug=False,
        enable_asserts=False,
        num_devices=8,
    )

    x16 = nc.dram_tensor("x16", [L, C], fp16, kind="ExternalInput").ap()
    wd = nc.dram_tensor("wd", [128, G * K * 128], fp16, kind="ExternalInput").ap()
    bb = nc.dram_tensor("bb", [128, G], fp32, kind="ExternalInput").ap()
    y16 = nc.dram_tensor("y16", [L, C], fp16, kind="ExternalOutput").ap()

    with TileContext(nc) as tc:
        with (
            tc.tile_pool(name="wpool", bufs=1) as wpool,
            tc.tile_pool(name="xpool", bufs=2) as xpool,
            tc.tile_pool(name="ypool", bufs=1) as ypool,
            tc.tile_pool(name="opool", bufs=4) as opool,
            tc.tile_pool(name="pspool", bufs=2, space="PSUM") as pspool,
        ):
            # Weights: [128, G*K*128] fp16, diag blocks; bias: [128, G] fp32.
            wtile = wpool.tile([128, G * K * 128], fp16, tag="w")
            nc.sync.dma_start(wtile[:], wd[:])
            btile = wpool.tile([128, G], fp32, tag="b")
            nc.sync.dma_start(btile[:], bb[:])

            ycl = {}
            for g in range(G):
                # x transposed: [128 ch, PAD + L] with causal zero pad.
                xt = xpool.tile([128, PAD + L], fp16, tag="xt")
                nc.vector.memset(xt[:, 0:PAD], 0.0)
                nc.sync.dma_start_transpose(
                    xt[:, PAD : PAD + L], x16[:, g * 128 : (g + 1) * 128]
                )
                for h in range(L // HALF):
                    ps = pspool.tile([128, HALF], fp32, tag="ps")
                    for j in range(K):
                        lhsT = wtile[:, (g * K + j) * 128 : (g * K + j + 1) * 128]
                        for n in range(HALF // NCHUNK):
                            l0 = h * HALF + n * NCHUNK
                            a = PAD - (K - 1) + l0 + j
                            nc.tensor.matmul(
                                ps[:, n * NCHUNK : (n + 1) * NCHUNK],
                                lhsT,
                                xt[:, a : a + NCHUNK],
                                start=(j == 0),
                                stop=(j == K - 1),
                            )
                    yt = ypool.tile([128, HALF], fp16, tag=f"y{g}_{h}")
                    nc.scalar.activation(
                        yt[:],
                        ps[:],
                        mybir.ActivationFunctionType.Identity,
                        bias=btile[:, g : g + 1],
                        scale=1.0,
                    )
                    ycl[(g, h)] = yt

            # Transpose back to [L, C] and store.
            for lb in range(L // 128):
                ot = opool.tile([128, C], fp16, tag="ot")
                h, off = (lb * 128) // HALF, (lb * 128) % HALF
                for g in range(G):
                    nc.sync.dma_start_transpose(
                        ot[:, g * 128 : (g + 1) * 128],
                        ycl[(g, h)][:, off : off + 128],
                    )
                nc.sync.dma_start(y16[lb * 128 : (lb + 1) * 128, :], ot[:])

    nc.compile()
    return nc


def _get_program():
    if "nc" not in _CACHE:
        _CACHE["nc"] = _build_device_program()
    return _CACHE["nc"]


def kernel(x, weight, bias):
    global LAST_RESULTS
    from concourse import bass_utils

    x = np.asarray(x)
    weight = np.asarray(weight)
    bias = np.asarray(bias)

    nc = _get_program()

    # Host-side prep: per-core batch shard (fp16) + replicated weights.
    # Diagonal blocks laid out exactly as the SBUF tile: [p, (g k m)].
    w = weight[:, 0, :].astype(np.float16)  # [C, K]
    wd4 = np.zeros((G, K, 128, 128), dtype=np.float16)
    idx = np.arange(128)
    for g in range(G):
        for j in range(K):
            wd4[g, j, idx, idx] = w[g * 128 : (g + 1) * 128, j]
    wd = np.ascontiguousarray(wd4.transpose(2, 0, 1, 3).reshape(128, G * K * 128))
    bb = np.ascontiguousarray(bias.astype(np.float32).reshape(G, 128).T)

    in_maps = []
    for b in range(B):
        in_maps.append(
            {
                "x16": np.ascontiguousarray(x[b]).astype(np.float16),
                "wd": wd,
                "bb": bb,
            }
        )

    trace = bool(int(os.environ.get("KERNEL_TRACE", "0")))
    res = bass_utils.run_bass_kernel_spmd(
        nc, in_maps, core_ids=list(range(B)), trace=trace
    )
    LAST_RESULTS = res
    _CACHE["last_in_maps"] = in_maps

    out = np.empty((B, L, C), dtype=np.float32)
    for b in range(B):
        out[b] = res.results[b]["y16"].astype(np.float32)
    return out
